# revision 31
# baseline (speedup 1.0000x reference)
"""LDA-loss logits kernel for Trainium2 (8 NeuronCores, SPMD).

Computes logits[b, c] = -0.5 * ||feat[b] - centers[c]||^2
                      = feat[b]·centers[c] - 0.5||feat[b]||^2 - 0.5||centers[c]||^2

Strategy (v4, fp8 DoubleRow at the silicon roofline):
  - Shard feat over batch: 4096 rows -> 512/core (4 m-tiles of 128), centers
    replicated.  Classes tiled 10000 = 19 n-tiles of 512 + one ragged tile
    of 272 -- no padded output columns, ~1.5us less PE work than padding to
    10240.  Wide moving operands hide LDWEIGHTS.
  - Inputs quantized to fp8e4 on host; matmuls run perf_mode=DoubleRow
    (2 contraction chunks of 128 per pass).  Measured 217ns per K=256/N=512
    matmul back-to-back = the fp8 peak; DoublePixel/DoubleColumn were probed
    on HW and are NOT faster for fp8 (same 512-cycle stream, less work).
    The squared-norm biases are host-precomputed fp32/fp16; fp8 error only
    touches the cross term (~6e-3 rel, tolerance 2e-2).
  - n-outer loop; center n-tiles stream in individually, and the first
    n-tile/feat are further split (per k-pair / per m-tile) so the first
    matmul issues after ~256KB of DMA instead of the full working set.
  - csq is host-broadcast to all 128 partitions and DMA'd in 4-tile blocks
    (on-chip GpSimd partition_broadcast was tried: it saves 2.6MB/core of
    HBM but the shared DVE<->GpSimd SBUF port slows eviction ADDs up to
    7.7x and the reshuffled prologue costs ~5us net -- five measured runs).
  - Eviction: ScalarE adds the per-row bias (fp32 PSUM -> fp16 SBUF),
    VectorE adds the per-column bias (fp16, 2x DVE rate).  The host upcasts
    the fp16 output after the gather (halves output HBM traffic).
  - All input DMA on the sync HWDGE ring in consumption order, outputs
    batched 4 m-tiles per store on the Activation ring.  Keep each class of
    traffic on one ring: any reassignment makes the tile scheduler reshuffle
    dispatch order (measured +1.5-4us of new stream stalls, four attempts).
  - Run-to-run exec noise is real: the device clock throttles 0-20% some
    runs (steady-state MM 216 -> 259ns) and chip-wide HBM contention from
    the 8 sibling cores adds 0-4us of stream gaps.
"""

import numpy as np
import ml_dtypes

BATCH = 4096
FEAT_DIM = 1024
NUM_CLASSES = 10000
N_CORES = 8
B_PER = BATCH // N_CORES            # 512 rows per core
P = 128
MT = B_PER // P                     # 4 output row tiles per core
KO = FEAT_DIM // P                  # 8 contraction chunks
KP = KO // 2                        # 4 DoubleRow chunk-pairs
NT = 20                             # n-tiles of 512 (last one ragged)
LAST = 272                          # last n-tile width: 10000 = 19*512 + 272
C_PAD = (NT - 1) * 512 + LAST       # = 10000, no padded output columns
C_FULL = NT * 512                   # 10240 padded layout for centsT/csq DRAM

_NC = None


def _build_bass():
    import concourse.mybir as mybir
    import concourse.tile as tile
    from concourse import bacc

    nc = bacc.Bacc("TRN2", target_bir_lowering=False, debug=False)

    featT = nc.dram_tensor("featT", [MT, P, KO * P], mybir.dt.float8e4,
                           kind="ExternalInput")
    centsT = nc.dram_tensor("centsT", [NT, P, KO * 512], mybir.dt.float8e4,
                            kind="ExternalInput")
    fsq = nc.dram_tensor("fsq", [P, MT], mybir.dt.float32, kind="ExternalInput")
    csq = nc.dram_tensor("csq", [NT // 4, P, 4 * 512], mybir.dt.float16,
                         kind="ExternalInput")
    out = nc.dram_tensor("out", [B_PER, C_PAD], mybir.dt.float16,
                         kind="ExternalOutput")  # C_PAD == 10000, no padding

    with tile.TileContext(nc) as tc:
        _lda_tile_kernel(tc, featT.ap(), centsT.ap(), fsq.ap(), csq.ap(),
                         out.ap())
    nc.compile()
    return nc


def _lda_tile_kernel(tc, featT, centsT, fsq, csq, out):
    import concourse.mybir as mybir

    nc = tc.nc
    out_r = out.rearrange("(mo p) c -> p mo c", p=P)

    with (
        tc.tile_pool(name="big", bufs=1) as big,
        tc.tile_pool(name="consts", bufs=1) as consts,
        tc.tile_pool(name="ostage", bufs=6) as ostage,
        tc.tile_pool(name="psum", bufs=8, space="PSUM") as psum,
    ):
        cent_sb = big.tile([P, NT, KO, 512], mybir.dt.float8e4)
        feat_sb = big.tile([P, MT, KO, P], mybir.dt.float8e4)
        csq_sb = consts.tile([P, NT, 512], mybir.dt.float16)
        fsq_sb = consts.tile([P, MT], mybir.dt.float32)
        warm_sb = consts.tile([P, 192], mybir.dt.float8e4)

        # All input loads on the sync HWDGE queue in consumption order.  The
        # first matmul needs only feat m-tile 0 + the first k-pair of center
        # n-tile 0 (~256KB), so split those loads fine-grained; everything
        # later goes in n-tile-sized chunks that stay ahead of compute.
        # (Splitting the prologue across both rings was measured twice: the
        # tile scheduler reshuffles dispatch order and adds 1-4us of new
        # mid-stream stalls -- keep every input load on one ring.)
        c0 = centsT[0].rearrange("p (ko c) -> p ko c", ko=KO)
        nc.sync.dma_start(feat_sb[:, 0],
                          featT[0].rearrange("p (ko f) -> p ko f", ko=KO))
        nc.sync.dma_start(cent_sb[:, 0, 0:2], c0[:, 0:2])
        for m in range(1, MT):
            nc.sync.dma_start(
                feat_sb[:, m], featT[m].rearrange("p (ko f) -> p ko f", ko=KO))
        for kp in range(1, KP):
            nc.sync.dma_start(cent_sb[:, 0, 2 * kp:2 * kp + 2],
                              c0[:, 2 * kp:2 * kp + 2])
        nc.sync.dma_start(fsq_sb[:], fsq)
        # Per-tile center loads: the tile framework signals completion
        # per-DMA, so coarser batches delay every tile in the batch to the
        # last byte and starve the matmul stream mid-flight (measured ~8us
        # of tensor gaps with 4-tile batches).  ~633ns of dispatch per DMA
        # is the cheaper side of that trade.  (Batching feat m1-3 / pulling
        # cents1 earlier was also measured: it shaved the prologue 0.7us but
        # introduced ~2.7us of new stream gaps -- net worse.)
        for j in range(1, NT):
            wj = LAST if j == NT - 1 else 512
            nc.sync.dma_start(
                cent_sb[:, j, :, 0:wj],
                centsT[j].rearrange("p (ko c) -> p ko c", ko=KO)[:, :, 0:wj])
            if j % 4 == 1:
                b = j // 4
                nc.sync.dma_start(
                    csq_sb[:, 4 * b:4 * b + 4],
                    csq[b].rearrange("p (j c) -> p j c", j=4))

        # PE warm-up: ~3us of throwaway matmuls during the DMA prologue so
        # the HAM clock gate opens (1.2 -> 2.4 GHz takes ~3.4us of sustained
        # PE activity) before the first real matmul issues.  The feed buffer
        # is memset on GpSimd (first engine to open its scope, ~0.9us before
        # Vector) rather than fed by a DMA: DMA-completion waits start the
        # warm-up ~3us LATE (measured).
        nc.gpsimd.memset(warm_sb[:], 0)
        warm_ps = psum.tile([P, 512], mybir.dt.float32, tag="ps", name="ps")
        for _ in range(56):
            nc.tensor.matmul(warm_ps[:, 0:64], warm_sb[:, 0:P],
                             warm_sb[:, P:P + 64], start=True, stop=True)

        for j in range(NT):
            W = LAST if j == NT - 1 else 512
            ps = [psum.tile([P, 512], mybir.dt.float32, tag="ps", name="ps")
                  for _ in range(MT)]
            for kp in range(KP):
                for m in range(MT):
                    nc.tensor.matmul(
                        ps[m][:, 0:W],
                        feat_sb[:, m, 2 * kp:2 * kp + 2, :],
                        cent_sb[:, j, 2 * kp:2 * kp + 2, 0:W],
                        start=(kp == 0),
                        stop=(kp == KP - 1),
                        perf_mode=mybir.MatmulPerfMode.DoubleRow,
                    )
            ot = ostage.tile([P, MT, 512], mybir.dt.float16, tag="ot",
                             name="ot")
            for m in range(MT):
                # ot[m] = psum + fsq[row]  (per-partition bias on ScalarE)
                nc.scalar.activation(
                    ot[:, m, 0:W], ps[m][:, 0:W],
                    mybir.ActivationFunctionType.Identity,
                    bias=fsq_sb[:, m:m + 1],
                )
                # ot[m] += csq[col]  (per-column bias on VectorE, fp16)
                nc.vector.tensor_add(ot[:, m, 0:W], ot[:, m, 0:W],
                                     csq_sb[:, j, 0:W])
            if j < NT - 1:
                # Output on the second HWDGE ring (Activation engine):
                # separate FIFO from the input ring, so stores never queue
                # behind loads.
                nc.scalar.dma_start(out_r[:, :, j * 512:(j + 1) * 512], ot)
            else:
                # Final n-tile: ONE batched store on the sync ring (idle by
                # now).  Per-m stores serialize four ~650ns descriptor
                # generations on the sync sequencer AFTER the evictions; a
                # single dispatch right after the last ADD lands the final
                # bytes ~1.1us earlier.  (Alternating rings here was measured
                # to reshuffle the PROLOGUE dispatch order, +2.7us -- don't.)
                nc.sync.dma_start(
                    out_r[:, :, j * 512:j * 512 + W], ot[:, :, 0:W])


def _get_nc():
    global _NC
    if _NC is None:
        _NC = _build_bass()
    return _NC


def _prep_inputs(feat, centers):
    feat = np.asarray(feat, dtype=np.float32)
    centers = np.asarray(centers, dtype=np.float32)
    f8 = ml_dtypes.float8_e4m3

    cent_pad = np.zeros((C_FULL, FEAT_DIM), dtype=np.float32)
    cent_pad[:NUM_CLASSES] = centers
    # centsT_sw[j, p, ko*512 + c] = centers[j*512 + c, ko*128 + p]
    centsT_sw = np.ascontiguousarray(
        cent_pad.T.astype(f8).reshape(KO, P, NT, 512).transpose(2, 1, 0, 3)
    ).reshape(NT, P, KO * 512)

    csq_v = np.zeros(C_FULL, dtype=np.float32)
    csq_v[:NUM_CLASSES] = -0.5 * np.einsum("cd,cd->c", centers, centers)
    csq_sw = np.ascontiguousarray(np.broadcast_to(
        csq_v.astype(np.float16).reshape(NT // 4, 1, 4 * 512),
        (NT // 4, P, 4 * 512)))

    feat8 = feat.astype(f8)
    fsq_v = -0.5 * np.einsum("bd,bd->b", feat, feat)

    in_maps = []
    for i in range(N_CORES):
        r0 = i * B_PER
        # featT_sw[mt, p, ko*128 + m] = feat[r0 + mt*128 + m, ko*128 + p]
        featT_sw = np.ascontiguousarray(
            feat8[r0:r0 + B_PER].T.reshape(KO, P, MT, P).transpose(2, 1, 0, 3)
        ).reshape(MT, P, KO * P)
        fsq_mat = np.ascontiguousarray(
            fsq_v[r0:r0 + B_PER].reshape(MT, P).T)
        in_maps.append({
            "featT": featT_sw,
            "centsT": centsT_sw,
            "fsq": fsq_mat,
            "csq": csq_sw,
        })
    return in_maps


def _run(inputs, trace=False, trace_cores=None):
    from concourse import bass_utils

    nc = _get_nc()
    in_maps = _prep_inputs(inputs["feat"], inputs["centers"])
    res = bass_utils.run_bass_kernel_spmd(
        nc, in_maps, core_ids=list(range(N_CORES)), trace=trace,
        trace_cores=trace_cores,
    )
    full = np.concatenate(
        [np.asarray(res.results[i]["out"]) for i in range(N_CORES)], axis=0)
    return full[:, :NUM_CLASSES].astype(np.float32), res


def kernel(**inputs) -> np.ndarray:
    return _run(inputs)[0]



# revision 33
# speedup vs baseline: 1.0125x; 1.0125x over previous
"""LDA-loss logits kernel for Trainium2 (8 NeuronCores, SPMD).

Computes logits[b, c] = -0.5 * ||feat[b] - centers[c]||^2
                      = feat[b]·centers[c] - 0.5||feat[b]||^2 - 0.5||centers[c]||^2

Strategy (v4, fp8 DoubleRow at the silicon roofline):
  - Shard feat over batch: 4096 rows -> 512/core (4 m-tiles of 128), centers
    replicated.  Classes tiled 10000 = 19 n-tiles of 512 + one ragged tile
    of 272 -- no padded output columns, ~1.5us less PE work than padding to
    10240.  Wide moving operands hide LDWEIGHTS.
  - Inputs quantized to fp8e4 on host; matmuls run perf_mode=DoubleRow
    (2 contraction chunks of 128 per pass).  Measured 217ns per K=256/N=512
    matmul back-to-back = the fp8 peak; DoublePixel/DoubleColumn were probed
    on HW and are NOT faster for fp8 (same 512-cycle stream, less work).
    The squared-norm biases are host-precomputed fp32/fp16; fp8 error only
    touches the cross term (~6e-3 rel, tolerance 2e-2).
  - n-outer loop; center n-tiles stream in individually, and the first
    n-tile/feat are further split (per k-pair / per m-tile) so the first
    matmul issues after ~256KB of DMA instead of the full working set.
  - csq is host-broadcast to all 128 partitions and DMA'd in 4-tile blocks
    (on-chip GpSimd partition_broadcast was tried: it saves 2.6MB/core of
    HBM but the shared DVE<->GpSimd SBUF port slows eviction ADDs up to
    7.7x and the reshuffled prologue costs ~5us net -- five measured runs).
  - Eviction: ScalarE adds the per-row bias (fp32 PSUM -> fp16 SBUF),
    VectorE adds the per-column bias (fp16, 2x DVE rate).  The host upcasts
    the fp16 output after the gather (halves output HBM traffic).
  - All input DMA on the sync HWDGE ring in consumption order, outputs
    batched 4 m-tiles per store on the Activation ring.  Keep each class of
    traffic on one ring: any reassignment makes the tile scheduler reshuffle
    dispatch order (measured +1.5-4us of new stream stalls, four attempts).
  - Run-to-run exec noise is real: the device clock throttles 0-20% some
    runs (steady-state MM 216 -> 259ns) and chip-wide HBM contention from
    the 8 sibling cores adds 0-4us of stream gaps.
"""

import numpy as np
import ml_dtypes

BATCH = 4096
FEAT_DIM = 1024
NUM_CLASSES = 10000
N_CORES = 8
B_PER = BATCH // N_CORES            # 512 rows per core
P = 128
MT = B_PER // P                     # 4 output row tiles per core
KO = FEAT_DIM // P                  # 8 contraction chunks
KP = KO // 2                        # 4 DoubleRow chunk-pairs
NT = 20                             # n-tiles of 512 (last one ragged)
LAST = 272                          # last n-tile width: 10000 = 19*512 + 272
C_PAD = (NT - 1) * 512 + LAST       # = 10000, no padded output columns
C_FULL = NT * 512                   # 10240 padded layout for centsT/csq DRAM

_NC = None


def _build_bass():
    import concourse.mybir as mybir
    import concourse.tile as tile
    from concourse import bacc

    nc = bacc.Bacc("TRN2", target_bir_lowering=False, debug=False)

    featT = nc.dram_tensor("featT", [MT, P, KO * P], mybir.dt.float8e4,
                           kind="ExternalInput")
    centsT = nc.dram_tensor("centsT", [NT, P, KO * 512], mybir.dt.float8e4,
                            kind="ExternalInput")
    fsq = nc.dram_tensor("fsq", [P, MT], mybir.dt.float32, kind="ExternalInput")
    csq = nc.dram_tensor("csq", [NT // 4, P, 4 * 512], mybir.dt.float16,
                         kind="ExternalInput")
    out = nc.dram_tensor("out", [B_PER, C_PAD], mybir.dt.float16,
                         kind="ExternalOutput")  # C_PAD == 10000, no padding

    with tile.TileContext(nc) as tc:
        _lda_tile_kernel(tc, featT.ap(), centsT.ap(), fsq.ap(), csq.ap(),
                         out.ap())
    nc.compile()
    return nc


def _lda_tile_kernel(tc, featT, centsT, fsq, csq, out):
    import concourse.mybir as mybir

    nc = tc.nc
    out_r = out.rearrange("(mo p) c -> p mo c", p=P)

    with (
        tc.tile_pool(name="big", bufs=1) as big,
        tc.tile_pool(name="consts", bufs=1) as consts,
        tc.tile_pool(name="ostage", bufs=6) as ostage,
        tc.tile_pool(name="psum", bufs=8, space="PSUM") as psum,
    ):
        cent_sb = big.tile([P, NT, KO, 512], mybir.dt.float8e4)
        feat_sb = big.tile([P, MT, KO, P], mybir.dt.float8e4)
        csq_sb = consts.tile([P, NT, 512], mybir.dt.float16)
        fsq_sb = consts.tile([P, MT], mybir.dt.float32)
        warm_sb = consts.tile([P, 192], mybir.dt.float8e4)

        # All input loads on the sync HWDGE queue in consumption order.  The
        # first matmul needs only feat m-tile 0 + the first k-pair of center
        # n-tile 0 (~256KB), so split those loads fine-grained; everything
        # later goes in n-tile-sized chunks that stay ahead of compute.
        # (Splitting the prologue across both rings was measured twice: the
        # tile scheduler reshuffles dispatch order and adds 1-4us of new
        # mid-stream stalls -- keep every input load on one ring.)
        c0 = centsT[0].rearrange("p (ko c) -> p ko c", ko=KO)
        nc.sync.dma_start(feat_sb[:, 0],
                          featT[0].rearrange("p (ko f) -> p ko f", ko=KO))
        nc.sync.dma_start(cent_sb[:, 0, 0:2], c0[:, 0:2])
        # Interleave feat m-tiles with cents0 k-pairs: the stream consumes
        # cents0 kp1/kp2/kp3 at +0.9/+1.7/+2.6us after its first matmul, so
        # dispatching all three feat tiles first (each ~633ns of ring
        # dispatch) made kp2/kp3 arrive late (measured 1.3us of j0-internal
        # stream gaps).
        for i in range(1, MT):
            nc.sync.dma_start(
                feat_sb[:, i], featT[i].rearrange("p (ko f) -> p ko f", ko=KO))
            nc.sync.dma_start(cent_sb[:, 0, 2 * i:2 * i + 2],
                              c0[:, 2 * i:2 * i + 2])
        nc.sync.dma_start(fsq_sb[:], fsq)
        # Per-tile center loads: the tile framework signals completion
        # per-DMA, so coarser batches delay every tile in the batch to the
        # last byte and starve the matmul stream mid-flight (measured ~8us
        # of tensor gaps with 4-tile batches).  ~633ns of dispatch per DMA
        # is the cheaper side of that trade.  (Batching feat m1-3 / pulling
        # cents1 earlier was also measured: it shaved the prologue 0.7us but
        # introduced ~2.7us of new stream gaps -- net worse.)
        for j in range(1, NT):
            wj = LAST if j == NT - 1 else 512
            nc.sync.dma_start(
                cent_sb[:, j, :, 0:wj],
                centsT[j].rearrange("p (ko c) -> p ko c", ko=KO)[:, :, 0:wj])
            # csq block b covers tiles 4b..4b+3; block 0 loads after tile 2
            # (not tile 1) so center tile 2 isn't delayed behind it right
            # when the stream is still catching up (measured 1.1us gap).
            # The j0 eviction ADD waits ~1us on block 0, but vector ADDs
            # don't gate the PSUM rotation -- only ScalarE evictions do.
            if j == 2 or (j % 4 == 1 and j > 4):
                b = (j - 1) // 4
                nc.sync.dma_start(
                    csq_sb[:, 4 * b:4 * b + 4],
                    csq[b].rearrange("p (j c) -> p j c", j=4))

        # PE warm-up: ~3us of throwaway matmuls during the DMA prologue so
        # the HAM clock gate opens (1.2 -> 2.4 GHz takes ~3.4us of sustained
        # PE activity) before the first real matmul issues.  The feed buffer
        # is memset on GpSimd (first engine to open its scope, ~0.9us before
        # Vector) rather than fed by a DMA: DMA-completion waits start the
        # warm-up ~3us LATE (measured).
        nc.gpsimd.memset(warm_sb[:], 0)
        warm_ps = psum.tile([P, 512], mybir.dt.float32, tag="ps", name="ps")
        for _ in range(56):
            nc.tensor.matmul(warm_ps[:, 0:64], warm_sb[:, 0:P],
                             warm_sb[:, P:P + 64], start=True, stop=True)

        for j in range(NT):
            W = LAST if j == NT - 1 else 512
            ps = [psum.tile([P, 512], mybir.dt.float32, tag="ps", name="ps")
                  for _ in range(MT)]
            for kp in range(KP):
                for m in range(MT):
                    nc.tensor.matmul(
                        ps[m][:, 0:W],
                        feat_sb[:, m, 2 * kp:2 * kp + 2, :],
                        cent_sb[:, j, 2 * kp:2 * kp + 2, 0:W],
                        start=(kp == 0),
                        stop=(kp == KP - 1),
                        perf_mode=mybir.MatmulPerfMode.DoubleRow,
                    )
            ot = ostage.tile([P, MT, 512], mybir.dt.float16, tag="ot",
                             name="ot")
            for m in range(MT):
                # ot[m] = psum + fsq[row]  (per-partition bias on ScalarE)
                nc.scalar.activation(
                    ot[:, m, 0:W], ps[m][:, 0:W],
                    mybir.ActivationFunctionType.Identity,
                    bias=fsq_sb[:, m:m + 1],
                )
                # ot[m] += csq[col]  (per-column bias on VectorE, fp16)
                nc.vector.tensor_add(ot[:, m, 0:W], ot[:, m, 0:W],
                                     csq_sb[:, j, 0:W])
            if j < NT - 1:
                # Output on the second HWDGE ring (Activation engine):
                # separate FIFO from the input ring, so stores never queue
                # behind loads.
                nc.scalar.dma_start(out_r[:, :, j * 512:(j + 1) * 512], ot)
            else:
                # Final n-tile: ONE batched store on the sync ring (idle by
                # now).  Per-m stores serialize four ~650ns descriptor
                # generations on the sync sequencer AFTER the evictions; a
                # single dispatch right after the last ADD lands the final
                # bytes ~1.1us earlier.  (Alternating rings here was measured
                # to reshuffle the PROLOGUE dispatch order, +2.7us -- don't.)
                nc.sync.dma_start(
                    out_r[:, :, j * 512:j * 512 + W], ot[:, :, 0:W])


def _get_nc():
    global _NC
    if _NC is None:
        _NC = _build_bass()
    return _NC


def _prep_inputs(feat, centers):
    feat = np.asarray(feat, dtype=np.float32)
    centers = np.asarray(centers, dtype=np.float32)
    f8 = ml_dtypes.float8_e4m3

    cent_pad = np.zeros((C_FULL, FEAT_DIM), dtype=np.float32)
    cent_pad[:NUM_CLASSES] = centers
    # centsT_sw[j, p, ko*512 + c] = centers[j*512 + c, ko*128 + p]
    centsT_sw = np.ascontiguousarray(
        cent_pad.T.astype(f8).reshape(KO, P, NT, 512).transpose(2, 1, 0, 3)
    ).reshape(NT, P, KO * 512)

    csq_v = np.zeros(C_FULL, dtype=np.float32)
    csq_v[:NUM_CLASSES] = -0.5 * np.einsum("cd,cd->c", centers, centers)
    csq_sw = np.ascontiguousarray(np.broadcast_to(
        csq_v.astype(np.float16).reshape(NT // 4, 1, 4 * 512),
        (NT // 4, P, 4 * 512)))

    feat8 = feat.astype(f8)
    fsq_v = -0.5 * np.einsum("bd,bd->b", feat, feat)

    in_maps = []
    for i in range(N_CORES):
        r0 = i * B_PER
        # featT_sw[mt, p, ko*128 + m] = feat[r0 + mt*128 + m, ko*128 + p]
        featT_sw = np.ascontiguousarray(
            feat8[r0:r0 + B_PER].T.reshape(KO, P, MT, P).transpose(2, 1, 0, 3)
        ).reshape(MT, P, KO * P)
        fsq_mat = np.ascontiguousarray(
            fsq_v[r0:r0 + B_PER].reshape(MT, P).T)
        in_maps.append({
            "featT": featT_sw,
            "centsT": centsT_sw,
            "fsq": fsq_mat,
            "csq": csq_sw,
        })
    return in_maps


def _run(inputs, trace=False, trace_cores=None):
    from concourse import bass_utils

    nc = _get_nc()
    in_maps = _prep_inputs(inputs["feat"], inputs["centers"])
    res = bass_utils.run_bass_kernel_spmd(
        nc, in_maps, core_ids=list(range(N_CORES)), trace=trace,
        trace_cores=trace_cores,
    )
    full = np.concatenate(
        [np.asarray(res.results[i]["out"]) for i in range(N_CORES)], axis=0)
    return full[:, :NUM_CLASSES].astype(np.float32), res


def kernel(**inputs) -> np.ndarray:
    return _run(inputs)[0]

